# revision 16
# baseline (speedup 1.0000x reference)
"""BiQRNN (fo-pooling) Trainium2 kernel, v4 — all-bf16 dataflow.

Data-parallel over batch across 8 NeuronCores (2 batch rows per core).
Per direction: g = W @ x with bf16 weights/activations (fp32 PSUM accum),
ACT tanh/sigmoid out of PSUM into bf16 gates, DVE tensor_tensor_scan
(fp32 internal state) for h_t = a_t*h_{t-1} + (1-a_t)*z_t, y = o*h on
GpSimd, Y stored bf16 and upcast on host. The backward direction runs
the same forward routine on a host-reversed copy of X.

Schedule notes:
- Weights are laid out hti-major (j' = hti*384 + g*128) so the first
  h-tile's three gates need only the first quarter of W; startup fetches
  W in per-(k, hti) pieces across the three DMA-capable queues and the
  PE stream starts as soon as the first quarter lands.
- Matmul/ACT run in T=1024 chunks (PSUM tile = 2 banks), but the gates
  accumulate into block-wide [128, 2048] tiles so the DVE runs ONE
  stt + ONE scan per (block, hti) — per-column scan cost is latency-bound
  (~2.5-3 cyc/elem regardless of dtype), so fewer/longer scans cut the
  DVE total by ~30%.
- The last block reverts to per-chunk scan chaining with a 1024/768/256
  taper so the post-matmul drain chain stays short.
"""

import numpy as np
from ml_dtypes import bfloat16

import concourse.bacc as bacc
import concourse.mybir as mybir
import concourse.tile as tile
from concourse import bass_utils

SEQ, BATCH, D_IN, HID = 2048, 16, 512, 512
NCORES = 8
BPC = BATCH // NCORES  # batch rows per core

f32 = mybir.dt.float32
bf16 = mybir.dt.bfloat16
Alu = mybir.AluOpType
Act = mybir.ActivationFunctionType

KT = D_IN // 128   # contraction tiles
HT = HID // 128    # h tiles per gate
MT = 3 * HT        # m tiles
T = 1024           # matmul/ACT chunk == rhs half-block tile
T0 = 256           # taper chunk at stream head/tail
WQ = 3 * 128       # weight columns per hti group (z,f,o)


def build_nc():
    nc = bacc.Bacc("TRN2", target_bir_lowering=False, debug=False)
    XT = nc.dram_tensor("xt", [2, KT, 128, BPC * SEQ], bf16, kind="ExternalInput")
    WT = nc.dram_tensor("wt", [2, KT, 128, 3 * HID], bf16, kind="ExternalInput")
    BIAS = nc.dram_tensor("bias", [2, 128, MT], f32, kind="ExternalInput")
    Y = nc.dram_tensor("y", [2, HT, 128, BPC * SEQ], bf16, kind="ExternalOutput")

    with tile.TileContext(nc) as tc:
        with (
            tc.tile_pool(name="wpool", bufs=1) as wpool,
            tc.tile_pool(name="bpool", bufs=1) as bpool,
            tc.tile_pool(name="rhs_pool", bufs=3) as rhs_pool,
            tc.tile_pool(name="ps_pool", bufs=4, space="PSUM") as ps_pool,
            tc.tile_pool(name="gate_pool", bufs=8) as gate_pool,
            tc.tile_pool(name="cp_pool", bufs=2) as cp_pool,
            tc.tile_pool(name="h_pool", bufs=4) as h_pool,
            tc.tile_pool(name="hl_pool", bufs=6) as hl_pool,
            tc.tile_pool(name="y_pool", bufs=2) as y_pool,
        ):
            w_sb = [[None] * KT for _ in range(2)]
            b_sb = [None, None]
            ENGS = (nc.sync, nc.gpsimd, nc.scalar)

            def load_w(d, k, eng):
                w = wpool.tile([128, 3 * HID], bf16, name=f"w_{d}_{k}")
                eng.dma_start(w[:], WT.ap()[d, k])
                w_sb[d][k] = w

            def load_bias(d, eng):
                bt = bpool.tile([128, MT], f32, name=f"b_{d}")
                eng.dma_start(bt[:], BIAS.ap()[d])
                b_sb[d] = bt

            def load_half(d, b, half, eng=None, split=False):
                """Fetch rhs half-block [128, KT, T] for (d, b, half)."""
                t = rhs_pool.tile([128, KT, T], bf16, name="rhs")
                c0 = b * SEQ + half * T
                if split:
                    # first chunk's (T0) columns land first, on 3 queues
                    for k in range(KT):
                        ENGS[k % 3].dma_start(
                            t[:, k, :T0], XT.ap()[d, k, :, c0 : c0 + T0]
                        )
                    for k in range(KT):
                        ENGS[k % 3].dma_start(
                            t[:, k, T0:], XT.ap()[d, k, :, c0 + T0 : c0 + T]
                        )
                else:
                    for k in range(KT):
                        (eng or nc.sync).dma_start(
                            t[:, k, :], XT.ap()[d, k, :, c0 : c0 + T]
                        )
                return t

            # --- startup ---
            # W d0 in per-(hti, k) pieces, hti-major, so the first h-tile's
            # gates (first WQ columns) land first; rhs chunk-0 heads race them.
            for d0k in range(KT):
                w = wpool.tile([128, 3 * HID], bf16, name=f"w_0_{d0k}")
                w_sb[0][d0k] = w
            for hti in range(HT):
                for k in range(KT):
                    ENGS[(hti * KT + k) % 3].dma_start(
                        w_sb[0][k][:, hti * WQ : (hti + 1) * WQ],
                        WT.ap()[0, k, :, hti * WQ : (hti + 1) * WQ],
                    )
                if hti == 0:
                    load_bias(0, nc.scalar)
            halves = [load_half(0, 0, 0, split=True), load_half(0, 0, 1, nc.gpsimd)]

            blocks = [(0, 0), (0, 1), (1, 0), (1, 1)]
            for bi, (d, b) in enumerate(blocks):
                last_block = bi == len(blocks) - 1
                if bi == 0:
                    chunks = [T0, T - T0, T]
                else:
                    chunks = [T, T - T0, T0] if last_block else [T, T]
                if bi + 1 < len(blocks):
                    dn, bn = blocks[bi + 1]
                    nxt = [load_half(dn, bn, 0), load_half(dn, bn, 1)]

                # block-wide bf16 gate accumulators (ACT writes per chunk)
                zts = [gate_pool.tile([128, SEQ], bf16, name="zt") for _ in range(HT)]
                ats = [gate_pool.tile([128, SEQ], bf16, name="at") for _ in range(HT)]
                ots = [gate_pool.tile([128, SEQ], bf16, name="ot") for _ in range(HT)]

                hprev = [None] * HT
                t0 = 0
                for ci, tl in enumerate(chunks):
                    if bi == 1:
                        # bw-direction constants trickle in while the fw
                        # stream runs; startup traffic has drained by now
                        for k in range(ci * 2, min(ci * 2 + 2, KT)):
                            load_w(1, k, nc.gpsimd)
                        if ci == 0:
                            load_bias(1, nc.scalar)
                    rhs = halves[t0 // T]
                    r0 = t0 % T
                    for hti in range(HT):
                        for g in range(3):
                            ps = ps_pool.tile([128, T], f32, name="ps")
                            wc = hti * WQ + g * 128
                            for s0 in range(0, tl, 512):
                                sl = min(512, tl - s0)
                                for k in range(KT):
                                    nc.tensor.matmul(
                                        ps[:, s0 : s0 + sl],
                                        w_sb[d][k][:, wc : wc + 128],
                                        rhs[:, k, r0 + s0 : r0 + s0 + sl],
                                        start=(k == 0),
                                        stop=(k == KT - 1),
                                    )
                            gt = (zts, ats, ots)[g][hti]
                            nc.scalar.activation(
                                gt[:, t0 : t0 + tl],
                                ps[:, :tl],
                                Act.Tanh if g == 0 else Act.Sigmoid,
                                bias=b_sb[d][:, hti * 3 + g : hti * 3 + g + 1],
                                scale=-1.0 if g == 1 else 1.0,
                            )
                        if last_block:
                            # per-chunk chaining keeps the drain chain short
                            zt, at, ot = zts[hti], ats[hti], ots[hti]
                            cp = cp_pool.tile([128, T], bf16, name="cpl")
                            nc.vector.scalar_tensor_tensor(
                                cp[:, :tl], at[:, t0 : t0 + tl], 1.0,
                                zt[:, t0 : t0 + tl],
                                op0=Alu.subtract, op1=Alu.mult,
                            )
                            h = hl_pool.tile([128, T], bf16, name="hl")
                            init = 0.0 if ci == 0 else hprev[hti]
                            nc.vector.tensor_tensor_scan(
                                h[:, :tl], at[:, t0 : t0 + tl], cp[:, :tl], init,
                                op0=Alu.mult, op1=Alu.subtract,
                            )
                            hprev[hti] = h[:, tl - 1 : tl]
                            yt = y_pool.tile([128, T], bf16, name="ytl")
                            nc.gpsimd.tensor_tensor(
                                yt[:, :tl], ot[:, t0 : t0 + tl], h[:, :tl],
                                op=Alu.mult,
                            )
                            nc.sync.dma_start(
                                Y.ap()[d, hti, :, b * SEQ + t0 : b * SEQ + t0 + tl],
                                yt[:, :tl],
                            )
                    t0 += tl

                if not last_block:
                    # one stt + one scan + one y per (block, hti) over the
                    # full 2048 columns — scans are latency-bound, so fewer
                    # longer scans cut DVE time
                    for hti in range(HT):
                        zt, at, ot = zts[hti], ats[hti], ots[hti]
                        cp = cp_pool.tile([128, SEQ], bf16, name="cp")
                        nc.vector.scalar_tensor_tensor(
                            cp[:], at[:], 1.0, zt[:],
                            op0=Alu.subtract, op1=Alu.mult,
                        )
                        h = h_pool.tile([128, SEQ], bf16, name="h")
                        nc.vector.tensor_tensor_scan(
                            h[:], at[:], cp[:], 0.0,
                            op0=Alu.mult, op1=Alu.subtract,
                        )
                        yt = y_pool.tile([128, SEQ], bf16, name="yt")
                        nc.gpsimd.tensor_tensor(yt[:], ot[:], h[:], op=Alu.mult)
                        nc.sync.dma_start(
                            Y.ap()[d, hti, :, b * SEQ : (b + 1) * SEQ], yt[:]
                        )
                    halves = nxt
    nc.compile()
    return nc


def prep_inputs(X, W_fw, b_fw, W_bw, b_bw):
    """Host-side shard/transpose/bf16-cast. Returns per-core in_maps."""
    # hti-major weight column order: j' = hti*384 + g*128 + c  <-  j = g*512 + hti*128 + c
    perm = np.empty(3 * HID, np.int64)
    for hti in range(HT):
        for g in range(3):
            j0 = g * HID + hti * 128
            perm[hti * WQ + g * 128 : hti * WQ + (g + 1) * 128] = np.arange(j0, j0 + 128)

    WTa = np.empty((2, KT, 128, 3 * HID), bfloat16)
    BIAS = np.empty((2, 128, MT), np.float32)
    for d, (W, bvec) in enumerate(((W_fw, b_fw), (W_bw, b_bw))):
        Wt = np.ascontiguousarray(W.T)[:, perm]  # [D, 3H] hti-major cols
        WTa[d] = Wt.reshape(KT, 128, 3 * HID).astype(bfloat16)
        bm = bvec.reshape(MT, 128)[perm[::128] // 128]  # reorder m rows hti-major
        bm = bm.T.copy()  # [128, MT]
        for hti in range(HT):
            bm[:, hti * 3 + 1] *= -1.0  # f-gate bias negated (a = sigmoid(-g - b))
        BIAS[d] = bm

    XTa = (
        np.ascontiguousarray(np.transpose(X, (2, 1, 0)))
        .astype(bfloat16)
        .reshape(KT, 128, BATCH, SEQ)
    )
    in_maps = []
    for c in range(NCORES):
        xt = np.empty((2, KT, 128, BPC, SEQ), bfloat16)
        blk = XTa[:, :, c * BPC : (c + 1) * BPC, :]
        xt[0] = blk
        xt[1] = blk[..., ::-1]
        in_maps.append(
            {"xt": xt.reshape(2, KT, 128, BPC * SEQ), "wt": WTa, "bias": BIAS}
        )
    return in_maps


def assemble_output(results):
    """results: list of per-core {'y': [2, HT, 128, tok]} -> [SEQ, BATCH, 2*HID]."""
    out = np.empty((SEQ, BATCH, 2 * HID), np.float32)
    for c in range(NCORES):
        Yc = np.asarray(results[c]["y"]).astype(np.float32)
        for b in range(BPC):
            gb = c * BPC + b
            yf = Yc[0, :, :, b * SEQ : (b + 1) * SEQ].reshape(HID, SEQ)
            yb = Yc[1, :, :, b * SEQ : (b + 1) * SEQ].reshape(HID, SEQ)
            out[:, gb, :HID] = yf.T
            out[:, gb, HID:] = yb.T[::-1]
    return out


_NC_CACHE = {}


def _get_nc():
    if "nc" not in _NC_CACHE:
        _NC_CACHE["nc"] = build_nc()
    return _NC_CACHE["nc"]


def kernel(X, W_fw, b_fw, W_bw, b_bw, trace=False):
    X = np.asarray(X, np.float32)
    nc = _get_nc()
    in_maps = prep_inputs(
        X,
        np.asarray(W_fw, np.float32),
        np.asarray(b_fw, np.float32),
        np.asarray(W_bw, np.float32),
        np.asarray(b_bw, np.float32),
    )
    res = bass_utils.run_bass_kernel_spmd(
        nc, in_maps, core_ids=list(range(NCORES)), trace=trace
    )
    out = assemble_output(res.results)
    if trace:
        kernel.last_results = res
    return out


# revision 17
# speedup vs baseline: 1.1198x; 1.1198x over previous
"""BiQRNN (fo-pooling) Trainium2 kernel, v5 — all-bf16 dataflow.

Data-parallel over batch across 8 NeuronCores (2 batch rows per core).
Per direction: g = W @ x with bf16 weights/activations (fp32 PSUM accum),
ACT tanh/sigmoid out of PSUM into bf16 gates, DVE tensor_tensor_scan
(fp32 internal state) for h_t = a_t*h_{t-1} + (1-a_t)*z_t chained across
T=1024 chunks, y = o*h on GpSimd, Y stored bf16 and upcast on host. The
backward direction runs the same forward routine on a host-reversed copy
of X. First/last chunks taper to 256 so the PE stream starts early and
the post-matmul drain chain stays short; the backward-direction weights
prefetch during the second block, after startup traffic has drained.
"""

import numpy as np
from ml_dtypes import bfloat16

import concourse.bacc as bacc
import concourse.mybir as mybir
import concourse.tile as tile
from concourse import bass_utils

SEQ, BATCH, D_IN, HID = 2048, 16, 512, 512
NCORES = 8
BPC = BATCH // NCORES  # batch rows per core

f32 = mybir.dt.float32
bf16 = mybir.dt.bfloat16
Alu = mybir.AluOpType
Act = mybir.ActivationFunctionType

KT = D_IN // 128   # contraction tiles
HT = HID // 128    # h tiles per gate
MT = 3 * HT        # m tiles
T = 1024           # matmul/ACT/scan chunk
T0 = 256           # taper chunk at stream head/tail


def build_nc():
    nc = bacc.Bacc("TRN2", target_bir_lowering=False, debug=False)
    XT = nc.dram_tensor("xt", [2, KT, 128, BPC * SEQ], bf16, kind="ExternalInput")
    WT = nc.dram_tensor("wt", [2, KT, 128, 3 * HID], bf16, kind="ExternalInput")
    BIAS = nc.dram_tensor("bias", [2, 128, MT], f32, kind="ExternalInput")
    Y = nc.dram_tensor("y", [2, HT, 128, BPC * SEQ], bf16, kind="ExternalOutput")

    with tile.TileContext(nc) as tc:
        with (
            tc.tile_pool(name="wpool", bufs=1) as wpool,
            tc.tile_pool(name="bpool", bufs=1) as bpool,
            tc.tile_pool(name="rhs_pool", bufs=2) as rhs_pool,
            tc.tile_pool(name="ps_pool", bufs=4, space="PSUM") as ps_pool,
            tc.tile_pool(name="gate_pool", bufs=12) as gate_pool,
            tc.tile_pool(name="h_pool", bufs=6) as h_pool,
            tc.tile_pool(name="y_pool", bufs=4) as y_pool,
        ):
            w_sb = [[None] * KT for _ in range(2)]
            b_sb = [None, None]

            def load_w(d, k, eng):
                w = wpool.tile([128, 3 * HID], bf16, name=f"w_{d}_{k}")
                eng.dma_start(w[:], WT.ap()[d, k])
                w_sb[d][k] = w

            def load_w_split3(d, k):
                w = wpool.tile([128, 3 * HID], bf16, name=f"w_{d}_{k}")
                q = 3 * HID // 3
                for p, eng in enumerate((nc.sync, nc.gpsimd, nc.scalar)):
                    eng.dma_start(
                        w[:, p * q : (p + 1) * q], WT.ap()[d, k, :, p * q : (p + 1) * q]
                    )
                w_sb[d][k] = w

            def load_bias(d, eng):
                bt = bpool.tile([128, MT], f32, name=f"b_{d}")
                eng.dma_start(bt[:], BIAS.ap()[d])
                b_sb[d] = bt

            def new_rhs():
                return rhs_pool.tile([128, KT, SEQ], bf16, name="rhs")

            def load_rhs(t, d, b, eng, k_lo=0, k_hi=KT, c0=0, c1=SEQ):
                for k in range(k_lo, k_hi):
                    eng.dma_start(
                        t[:, k, c0:c1], XT.ap()[d, k, :, b * SEQ + c0 : b * SEQ + c1]
                    )

            # --- startup: first-chunk deps first, spread across queues ---
            rhs0 = new_rhs()
            load_rhs(rhs0, 0, 0, nc.sync, k_lo=0, k_hi=1, c1=T0)
            load_rhs(rhs0, 0, 0, nc.gpsimd, k_lo=1, k_hi=2, c1=T0)
            load_rhs(rhs0, 0, 0, nc.scalar, k_lo=2, k_hi=3, c1=T0)
            load_rhs(rhs0, 0, 0, nc.gpsimd, k_lo=3, k_hi=4, c1=T0)
            load_w(0, 0, nc.sync)
            load_w(0, 1, nc.gpsimd)
            load_w(0, 2, nc.scalar)
            load_w_split3(0, 3)
            load_bias(0, nc.scalar)
            load_rhs(rhs0, 0, 0, nc.sync, k_lo=0, k_hi=1, c0=T0)
            load_rhs(rhs0, 0, 0, nc.gpsimd, k_lo=1, k_hi=2, c0=T0)
            load_rhs(rhs0, 0, 0, nc.scalar, k_lo=2, k_hi=3, c0=T0)
            load_rhs(rhs0, 0, 0, nc.gpsimd, k_lo=3, k_hi=4, c0=T0)

            rhs_next = [None]
            blocks = [(0, 0), (0, 1), (1, 0), (1, 1)]
            for bi, (d, b) in enumerate(blocks):
                last_block = bi == len(blocks) - 1
                if bi == 0:
                    rhs = rhs0
                    chunks = [T0, SEQ - T - T0, T]
                else:
                    rhs = rhs_next[0]
                    chunks = [T, SEQ - T - T0, T0] if last_block else [T, SEQ - T]
                if bi + 1 < len(blocks):
                    dn, bn = blocks[bi + 1]
                    rhs_next[0] = new_rhs()
                    load_rhs(rhs_next[0], dn, bn, nc.sync)

                hprev = [None] * HT
                t0 = 0
                for ci, tl in enumerate(chunks):
                    if bi == 1:
                        # bw-direction constants trickle in while the fw
                        # stream runs; startup traffic has drained by now
                        for k in range(ci * 2, min(ci * 2 + 2, KT)):
                            load_w(1, k, nc.gpsimd)
                        if ci == 0:
                            load_bias(1, nc.scalar)
                    for hti in range(HT):
                        acts = []
                        for g in range(3):
                            m = g * HT + hti
                            ps = ps_pool.tile([128, T], f32, name="ps")
                            for s0 in range(0, tl, 512):
                                sl = min(512, tl - s0)
                                for k in range(KT):
                                    nc.tensor.matmul(
                                        ps[:, s0 : s0 + sl],
                                        w_sb[d][k][:, m * 128 : (m + 1) * 128],
                                        rhs[:, k, t0 + s0 : t0 + s0 + sl],
                                        start=(k == 0),
                                        stop=(k == KT - 1),
                                    )
                            gt = gate_pool.tile(
                                [128, T], bf16, name=("zt", "at", "ot")[g]
                            )
                            nc.scalar.activation(
                                gt[:, :tl],
                                ps[:, :tl],
                                Act.Tanh if g == 0 else Act.Sigmoid,
                                bias=b_sb[d][:, m : m + 1],
                                scale=-1.0 if g == 1 else 1.0,
                            )
                            acts.append(gt)
                        zt, at, ot = acts
                        cp = gate_pool.tile([128, T], bf16, name="cp")
                        # cp = (a - 1) * z = -c
                        nc.vector.scalar_tensor_tensor(
                            cp[:, :tl], at[:, :tl], 1.0, zt[:, :tl],
                            op0=Alu.subtract, op1=Alu.mult,
                        )
                        h = h_pool.tile([128, T], bf16, name="h")
                        init = 0.0 if ci == 0 else hprev[hti]
                        # h_t = a_t * h_{t-1} - cp_t
                        nc.vector.tensor_tensor_scan(
                            h[:, :tl], at[:, :tl], cp[:, :tl], init,
                            op0=Alu.mult, op1=Alu.subtract,
                        )
                        hprev[hti] = h[:, tl - 1 : tl]
                        yt = y_pool.tile([128, T], bf16, name="yt")
                        nc.gpsimd.tensor_tensor(
                            yt[:, :tl], ot[:, :tl], h[:, :tl], op=Alu.mult
                        )
                        nc.sync.dma_start(
                            Y.ap()[d, hti, :, b * SEQ + t0 : b * SEQ + t0 + tl],
                            yt[:, :tl],
                        )
                    t0 += tl
    nc.compile()
    return nc


def prep_inputs(X, W_fw, b_fw, W_bw, b_bw):
    """Host-side shard/transpose/bf16-cast. Returns per-core in_maps."""
    WTa = np.empty((2, KT, 128, 3 * HID), bfloat16)
    BIAS = np.empty((2, 128, MT), np.float32)
    for d, (W, bvec) in enumerate(((W_fw, b_fw), (W_bw, b_bw))):
        WTa[d] = np.ascontiguousarray(W.T).reshape(KT, 128, 3 * HID).astype(bfloat16)
        bm = bvec.reshape(MT, 128).T.copy()  # [128, MT]
        bm[:, HT : 2 * HT] *= -1.0  # f-gate bias negated (a = sigmoid(-g - b))
        BIAS[d] = bm

    XTa = (
        np.ascontiguousarray(np.transpose(X, (2, 1, 0)))
        .astype(bfloat16)
        .reshape(KT, 128, BATCH, SEQ)
    )
    in_maps = []
    for c in range(NCORES):
        xt = np.empty((2, KT, 128, BPC, SEQ), bfloat16)
        blk = XTa[:, :, c * BPC : (c + 1) * BPC, :]
        xt[0] = blk
        xt[1] = blk[..., ::-1]
        in_maps.append(
            {"xt": xt.reshape(2, KT, 128, BPC * SEQ), "wt": WTa, "bias": BIAS}
        )
    return in_maps


def assemble_output(results):
    """results: list of per-core {'y': [2, HT, 128, tok]} -> [SEQ, BATCH, 2*HID]."""
    out = np.empty((SEQ, BATCH, 2 * HID), np.float32)
    for c in range(NCORES):
        Yc = np.asarray(results[c]["y"]).astype(np.float32)
        for b in range(BPC):
            gb = c * BPC + b
            yf = Yc[0, :, :, b * SEQ : (b + 1) * SEQ].reshape(HID, SEQ)
            yb = Yc[1, :, :, b * SEQ : (b + 1) * SEQ].reshape(HID, SEQ)
            out[:, gb, :HID] = yf.T
            out[:, gb, HID:] = yb.T[::-1]
    return out


_NC_CACHE = {}


def _get_nc():
    if "nc" not in _NC_CACHE:
        _NC_CACHE["nc"] = build_nc()
    return _NC_CACHE["nc"]


def kernel(X, W_fw, b_fw, W_bw, b_bw, trace=False):
    X = np.asarray(X, np.float32)
    nc = _get_nc()
    in_maps = prep_inputs(
        X,
        np.asarray(W_fw, np.float32),
        np.asarray(b_fw, np.float32),
        np.asarray(W_bw, np.float32),
        np.asarray(b_bw, np.float32),
    )
    res = bass_utils.run_bass_kernel_spmd(
        nc, in_maps, core_ids=list(range(NCORES)), trace=trace
    )
    out = assemble_output(res.results)
    if trace:
        kernel.last_results = res
    return out
